# revision 15
# baseline (speedup 1.0000x reference)
"""Trainium2 Bass kernel for causal GQA attention block (dense transformer).

Full module: qkv = clip(x @ Wqkv.T, +-8); half-split RoPE on q,k;
GQA causal attention (32 q heads, 8 kv heads, head_dim 128); out @ Wout.T.

Sharding: tensor-parallel over heads across 8 cores. Each core owns 4 q
heads + their shared kv head (rows of Wqkv) and the matching 512 columns
of Wout; it computes a full-shape partial of the output projection and the
host sums the 8 partials.

v2 restructure vs baseline:
- Attention inner loop processes HEAD PAIRS with a one-k-tile software
  pipeline lag: the PE emits [sc_a, sc_b, pv_a', sm_a', pv_b', sm_b'] per
  k-tile so it never stalls waiting for the Activation engine's exp.
- Diagonal (masked) k-tiles are interleaved with off-diagonal ones so the
  short PE blocks they produce sit next to full-width blocks.
- q/k/v/ex in bf16: every matmul's moving operand is bf16 (1 cycle/row at
  any width; fp32r drops to 4 cycles/row under 256-wide streams).
- Softmax row-sums for all 4 heads accumulate into one [4, CH] PSUM bank.
- Output projection accumulates 1024-wide column pairs in a 2-bank PSUM
  tile from the score-tile rotation; partial outputs are stored bf16
  (halves the output DMA; host sums partials in fp32).
- Weight DMAs interleave with the first x-tile DMAs to cut startup idle.
"""
import os
import sys
import math

for _p in ("/opt/trn_rl_repo", "/root/.axon_site/_ro/trn_rl_repo"):
    if os.path.isdir(_p) and _p not in sys.path:
        sys.path.insert(0, _p)

import numpy as np

import concourse.bass as bass
import concourse.tile as tile
from concourse import bacc, mybir
from concourse import bass_utils

# If BASS_TRACE is set in the environment, run_bass_kernel_spmd imports
# antenv.axon_hooks, which this image's antenv package lacks. Register a
# stub so tracing degrades gracefully instead of crashing.
try:
    import antenv.axon_hooks  # noqa: F401
except ImportError:
    try:
        import types
        import antenv

        _hooks = types.ModuleType("antenv.axon_hooks")
        _hooks._hook = None
        _hooks.set_axon_ntff_profile_hook = (
            lambda h: setattr(_hooks, "_hook", h))
        _hooks.get_axon_ntff_profile_hook = lambda: _hooks._hook
        sys.modules["antenv.axon_hooks"] = _hooks
        antenv.axon_hooks = _hooks
    except Exception:  # noqa: BLE001
        pass

F32 = mybir.dt.float32
F32R = mybir.dt.float32r
BF16 = mybir.dt.bfloat16
AF = mybir.ActivationFunctionType
OP = mybir.AluOpType

NEG_BIG = -1.0e30


def default_cfg():
    return dict(
        B=2, L=2048, D=4096, QH=4, HD=128,
        CH=512,   # attention q-chunk width == QN
        QN=512,   # qkv projection l-chunk width
        CLIP=8.0, theta=500000.0, ncores=8,
    )


def mini_cfg():
    return dict(
        B=2, L=1024, D=512, QH=4, HD=128,
        CH=512, QN=512,
        CLIP=8.0, theta=500000.0, ncores=1,
    )


def build_program(cfg):
    B, L, D = cfg["B"], cfg["L"], cfg["D"]
    QH, HD = cfg["QH"], cfg["HD"]
    CH, QN = cfg["CH"], cfg["QN"]
    CLIP = cfg["CLIP"]
    assert CH == QN
    RT = QH + 2               # r-tiles per core: QH q heads, k, v
    R = RT * HD
    CT = D // HD              # contraction tiles
    TPC = CH // HD            # k-tiles per attention chunk
    NJ = L // CH              # attention q-chunks per batch
    NLC = L // QN
    OCW = min(2 * CH, D)      # out-proj column group per PSUM tile
    scale = 1.0 / math.sqrt(HD)

    nc = bacc.Bacc("TRN2", target_bir_lowering=False, debug=False,
                   enable_asserts=True, num_devices=1)

    xT_d = nc.dram_tensor("xT", [D, B * L], F32R, kind="ExternalInput").ap()
    wq_d = nc.dram_tensor("wq", [D, R], F32R, kind="ExternalInput").ap()
    wo_d = nc.dram_tensor("wo", [QH * HD, D], BF16, kind="ExternalInput").ap()
    ra_d = nc.dram_tensor("ropeA", [HD, L], F32, kind="ExternalInput").ap()
    rb_d = nc.dram_tensor("ropeB", [HD, L], F32, kind="ExternalInput").ap()
    pm_d = nc.dram_tensor("perm", [HD, HD], F32R, kind="ExternalInput").ap()
    ms_d = nc.dram_tensor("mask", [HD, HD], F32, kind="ExternalInput").ap()
    o1_d = nc.dram_tensor("ones1", [1, HD], F32R, kind="ExternalInput").ap()
    ok_d = nc.dram_tensor("onesk", [HD, 1], BF16, kind="ExternalInput").ap()
    id_d = nc.dram_tensor("ident", [HD, HD], F32R, kind="ExternalInput").ap()
    out_d = nc.dram_tensor("out", [B * L, D], BF16, kind="ExternalOutput").ap()

    from contextlib import ExitStack
    with tile.TileContext(nc) as tc, ExitStack() as _es:
        wq_pool = _es.enter_context(tc.tile_pool(name="wq_pool", bufs=1))
        wo_pool = _es.enter_context(tc.tile_pool(name="wo_pool", bufs=1))
        cpool = _es.enter_context(tc.tile_pool(name="const", bufs=1))
        ex_pool = _es.enter_context(tc.tile_pool(name="ex", bufs=2))
        at_pool = _es.enter_context(tc.tile_pool(name="at", bufs=2 * QH + 1))
        bc_pool = _es.enter_context(tc.tile_pool(name="bc", bufs=1))
        rc_pool = _es.enter_context(tc.tile_pool(name="rc", bufs=1))
        fo_pool = _es.enter_context(tc.tile_pool(name="fo", bufs=2))
        # PSUM: 8 banks total.
        # P2A 2x[128,2CH] (4 banks): qkv acc0-3 / attn score pairs / bcp / fn
        # P2B 1x[128,2CH] (2 banks): qkv acc4-5 / attn pv pairs
        # P1A 1x[128,CH]  (1 bank):  qkv vtr + rot alt / attn sm4
        # P1C 1x[128,CH]  (1 bank):  qkv rot
        P2A = _es.enter_context(tc.tile_pool(name="P2A", bufs=2, space="PSUM"))
        P2B = _es.enter_context(tc.tile_pool(name="P2B", bufs=1, space="PSUM"))
        P1A = _es.enter_context(tc.tile_pool(name="P1A", bufs=1, space="PSUM"))
        P1C = _es.enter_context(tc.tile_pool(name="P1C", bufs=1, space="PSUM"))

        t_perm = cpool.tile([HD, HD], F32R, tag="t_perm")
        t_mask = cpool.tile([HD, HD], F32, tag="t_mask")
        t_ones1 = cpool.tile([1, HD], F32R, tag="t_ones1")
        t_onesk = cpool.tile([HD, 1], BF16, tag="t_onesk")
        t_ident = cpool.tile([HD, HD], F32R, tag="t_ident")
        nc.sync.dma_start(t_perm[:], pm_d[:])
        nc.sync.dma_start(t_mask[:], ms_d[:])
        nc.sync.dma_start(t_ones1[:], o1_d[:])
        nc.sync.dma_start(t_onesk[:], ok_d[:])
        nc.sync.dma_start(t_ident[:], id_d[:])

        # resident qkv weight tiles; DMAs for them are issued interleaved
        # with the first batch's x-tile DMAs (see qkv loop) so the PE can
        # start ~2us in instead of waiting for the full 12MB.
        w_ci = [wq_pool.tile([HD, R], F32R, tag=f"w{ci}", name=f"w{ci}")
                for ci in range(CT)]
        wo_sb = wo_pool.tile([HD, QH * D], BF16, tag="wo_sb")

        for b in range(B):
            with ExitStack() as _bs:
                qkv_pool = _bs.enter_context(
                    tc.tile_pool(name=f"qkv{b}", bufs=1))
                q_t = [[qkv_pool.tile([HD, QN], BF16, tag=f"q{h}_{lc}",
                                      name=f"q{h}_{b}_{lc}")
                        for lc in range(NLC)]
                       for h in range(QH)]
                k_t = [qkv_pool.tile([HD, QN], BF16, tag=f"k_{lc}",
                                     name=f"k_{b}_{lc}")
                       for lc in range(NLC)]
                v_t = [qkv_pool.tile([HD, QN], BF16, tag=f"v_{lc}",
                                     name=f"v_{b}_{lc}")
                       for lc in range(NLC)]

                # ---------------- qkv projection phase ----------------
                x_pool = _bs.enter_context(tc.tile_pool(name=f"xp{b}", bufs=4))
                cl_pool = _bs.enter_context(tc.tile_pool(name=f"cl{b}", bufs=1))
                rt_pool = _bs.enter_context(tc.tile_pool(name=f"rt{b}", bufs=2))
                tb_pool = _bs.enter_context(tc.tile_pool(name=f"tb{b}", bufs=1))

                def build_rope_groups(lc, cls, t_ra, t_rb):
                    """Deferred per-r rope/transpose emitters for chunk lc.

                    Emitted one group per ci-block of the NEXT chunk so the
                    rot matmuls never stall the PE on the DVE clip chain."""
                    groups = []
                    for r in range(QH + 1):
                        def g(r=r, lc=lc, cls=cls, t_ra=t_ra, t_rb=t_rb):
                            rpool = P1C if (r % 2 == 0) else P1A
                            rot = rpool.tile([HD, QN], F32, tag="p",
                                             name=f"rot{b}_{lc}_{r}")
                            nc.tensor.matmul(rot[:], t_perm[:], cls[r][:],
                                             start=True, stop=True)
                            t1 = rt_pool.tile([HD, QN], F32, tag="tmp")
                            nc.vector.tensor_tensor(
                                t1[:], rot[:], t_rb[:], OP.mult)
                            d2 = rt_pool.tile([HD, QN], F32, tag="tmp")
                            nc.vector.tensor_tensor(
                                d2[:], cls[r][:].bitcast(F32), t_ra[:],
                                OP.mult)
                            dest = (q_t[r][lc] if r < QH else k_t[lc])[:]
                            nc.vector.tensor_tensor(dest, d2[:], t1[:],
                                                    OP.add)
                        groups.append(g)

                    def gv(lc=lc, cls=cls):
                        vtr = P1A.tile([HD, QN], F32R, tag="p",
                                       name=f"vtr{b}_{lc}")
                        nt = QN // HD
                        for t in range(nt):
                            nc.tensor.matmul(
                                vtr[:, t * HD:(t + 1) * HD],
                                cls[QH + 1][:, t * HD:(t + 1) * HD],
                                t_ident[:],
                                is_transpose=True,
                                start=(t == 0), stop=(t == nt - 1))
                        nc.vector.tensor_copy(v_t[lc][:], vtr[:])
                    groups.append(gv)
                    return groups

                prev_rope = []
                for lc in range(NLC):
                    lsl = slice(lc * QN, (lc + 1) * QN)
                    t_ra = tb_pool.tile([HD, QN], F32, tag="t_ra",
                                        name=f"t_ra{b}_{lc}")
                    t_rb = tb_pool.tile([HD, QN], F32, tag="t_rb",
                                        name=f"t_rb{b}_{lc}")
                    nc.sync.dma_start(t_ra[:], ra_d[:, lsl])
                    nc.sync.dma_start(t_rb[:], rb_d[:, lsl])
                    accA = P2A.tile([HD, 2 * QN], F32, tag="p",
                                    name=f"accA{b}_{lc}")
                    accB = P2A.tile([HD, 2 * QN], F32, tag="p",
                                    name=f"accB{b}_{lc}")
                    accC = P2B.tile([HD, 2 * QN], F32, tag="p",
                                    name=f"accC{b}_{lc}")
                    acc = [accA[:, 0:QN], accA[:, QN:],
                           accB[:, 0:QN], accB[:, QN:],
                           accC[:, 0:QN], accC[:, QN:]][:RT]
                    for ci in range(CT):
                        xt = x_pool.tile([HD, QN], F32R, tag="xt")
                        nc.sync.dma_start(
                            xt[:],
                            xT_d[ci * HD:(ci + 1) * HD,
                                 b * L + lc * QN:b * L + (lc + 1) * QN])
                        if b == 0 and lc == 0:
                            nc.sync.dma_start(
                                w_ci[ci][:],
                                wq_d[ci * HD:(ci + 1) * HD, :])
                        for r in range(RT):
                            nc.tensor.matmul(
                                acc[r],
                                w_ci[ci][:, r * HD:(r + 1) * HD],
                                xt[:],
                                start=(ci == 0), stop=(ci == CT - 1))
                        if ci < len(prev_rope):
                            prev_rope[ci]()
                    for g in prev_rope[CT:]:
                        g()
                    cls = []
                    for r in range(RT):
                        cl = cl_pool.tile([HD, QN], F32R, tag=f"cl{r}",
                                          name=f"cl{b}_{lc}_{r}")
                        nc.vector.tensor_scalar(
                            cl[:], acc[r], -CLIP, CLIP, OP.max, OP.min)
                        cls.append(cl)
                    prev_rope = build_rope_groups(lc, cls, t_ra, t_rb)
                for g in prev_rope:
                    g()

                if b == 0:
                    for i in range(QH):
                        nc.sync.dma_start(wo_sb[:, i * D:(i + 1) * D],
                                          wo_d[i * HD:(i + 1) * HD, :])

                # ---------------- attention + out projection ----------
                def emit_fin(j, at_tiles):
                    for lt in range(TPC):
                        for ocp in range(D // OCW):
                            fn = P2A.tile([HD, OCW], F32, tag="p",
                                          name=f"fn{b}_{j}_{lt}_{ocp}")
                            for half in range(OCW // 512):
                                oc0 = ocp * OCW + half * 512
                                for i in range(QH):
                                    nc.tensor.matmul(
                                        fn[:, half * 512:(half + 1) * 512],
                                        at_tiles[i][:, lt * HD:(lt + 1) * HD],
                                        wo_sb[:, i * D + oc0:
                                              i * D + oc0 + 512],
                                        start=(i == 0), stop=(i == QH - 1))
                            fo = fo_pool.tile([HD, OCW], BF16, tag="fo")
                            if (lt + ocp) % 2 == 0:
                                nc.vector.tensor_copy(fo[:], fn[:])
                            else:
                                nc.scalar.copy(fo[:], fn[:])
                            row0 = b * L + j * CH + lt * HD
                            nc.sync.dma_start(
                                out_d[row0:row0 + HD,
                                      ocp * OCW:(ocp + 1) * OCW],
                                fo[:])

                prev_fin = None
                for j in range(NJ):
                    at_j = []
                    # diag k-tiles interleaved with off-diag so the short
                    # PE blocks sit next to full-width ones
                    diag_kis = [j * TPC + t for t in range(TPC)]
                    off_kis = list(range(j * TPC))
                    order = []
                    oi = 0
                    for dki in diag_kis:
                        order.append(dki)
                        if oi < len(off_kis):
                            order.append(off_kis[oi])
                            oi += 1
                    order.extend(off_kis[oi:])
                    nk = len(order)

                    for p in range(QH // 2):
                        ha = 2 * p
                        pvt = P2B.tile([HD, 2 * CH], F32, tag="p",
                                       name=f"pv{b}_{j}_{p}")
                        sm_ab = [P1A.tile([1, CH], F32, tag="p",
                                          name=f"sma{b}_{j}_{p}"),
                                 P1C.tile([1, CH], F32, tag="p",
                                          name=f"smb{b}_{j}_{p}")]

                        def emit_pvsm(pend):
                            (ext, w0, W, klc, kof, st, sp) = pend
                            for hh in range(2):
                                nc.tensor.matmul(
                                    pvt[:, hh * CH + w0:hh * CH + w0 + W],
                                    v_t[klc][:, kof:kof + HD],
                                    ext[:, hh * CH:hh * CH + W],
                                    start=st, stop=sp)
                                nc.tensor.matmul(
                                    sm_ab[hh][0:1, w0:w0 + W],
                                    t_onesk[:],
                                    ext[:, hh * CH:hh * CH + W],
                                    start=st, stop=sp)

                        pend = None
                        for idx, ki in enumerate(order):
                            diag = ki >= j * TPC
                            t = ki - j * TPC
                            w0 = t * HD if diag else 0
                            W = CH - w0
                            klc, kof = divmod(ki * HD, QN)
                            sct = P2A.tile([HD, 2 * CH], F32, tag="p",
                                           name=f"sc{b}_{j}_{p}_{ki}")
                            for hh in range(2):
                                nc.tensor.matmul(
                                    sct[:, hh * CH:hh * CH + W],
                                    k_t[klc][:, kof:kof + HD],
                                    q_t[ha + hh][j][:, w0:w0 + W],
                                    start=True, stop=True)
                            ext = ex_pool.tile([HD, 2 * CH], BF16, tag="ex")
                            if diag:
                                for hh in range(2):
                                    nc.vector.tensor_tensor(
                                        sct[:, hh * CH:hh * CH + HD],
                                        sct[:, hh * CH:hh * CH + HD],
                                        t_mask[:], OP.add)
                                    nc.scalar.activation(
                                        ext[:, hh * CH:hh * CH + W],
                                        sct[:, hh * CH:hh * CH + W],
                                        AF.Exp, scale=scale)
                            else:
                                nc.scalar.activation(
                                    ext[:, 0:CH + W], sct[:, 0:CH + W],
                                    AF.Exp, scale=scale)
                            if pend is not None:
                                emit_pvsm(pend)
                            pend = (ext, w0, W, klc, kof,
                                    idx == 0, idx == nk - 1)
                        emit_pvsm(pend)

                        # normalization for this head pair
                        bcp = P2A.tile([HD, 2 * CH], F32, tag="p",
                                       name=f"bcp{b}_{j}_{p}")
                        for hh in range(2):
                            rc32 = rc_pool.tile([1, CH], F32, tag="rc32",
                                                name=f"rc32_{b}_{j}_{p}{hh}")
                            nc.vector.reciprocal_approx_fast(
                                rc32[:], sm_ab[hh][:])
                            rc = rc_pool.tile([1, CH], F32R, tag="rc",
                                              name=f"rc_{b}_{j}_{p}{hh}")
                            nc.vector.tensor_copy(rc[:], rc32[:])
                            nc.tensor.matmul(bcp[:, hh * CH:hh * CH + CH],
                                             t_ones1[:], rc[:],
                                             start=True, stop=True)
                        bcs = bc_pool.tile([HD, 2 * CH], F32, tag="bcs")
                        nc.scalar.copy(bcs[:], bcp[:])
                        for hh in range(2):
                            at = at_pool.tile([HD, CH], BF16, tag="at",
                                              name=f"at{b}_{j}_{ha + hh}")
                            nc.vector.tensor_tensor(
                                at[:], pvt[:, hh * CH:(hh + 1) * CH],
                                bcs[:, hh * CH:(hh + 1) * CH], OP.mult)
                            at_j.append(at)

                    if prev_fin is not None:
                        emit_fin(*prev_fin)
                    prev_fin = (j, at_j)
                emit_fin(*prev_fin)
    nc.compile()
    return nc


def host_tables(cfg):
    import ml_dtypes
    L, HD, theta = cfg["L"], cfg["HD"], cfg["theta"]
    half = HD // 2
    inv_freq = 1.0 / (theta ** (np.arange(half, dtype=np.float64) / half))
    ang = np.arange(L, dtype=np.float64)[:, None] * inv_freq[None, :]  # [L,half]
    cos = np.cos(ang).astype(np.float32)   # [L, half]
    sin = np.sin(ang).astype(np.float32)
    ropeA = np.empty((HD, L), dtype=np.float32)
    ropeB = np.empty((HD, L), dtype=np.float32)
    ropeA[:half] = cos.T
    ropeA[half:] = cos.T
    ropeB[:half] = sin.T
    ropeB[half:] = sin.T

    perm = np.zeros((HD, HD), dtype=np.float32)
    for d in range(half):
        perm[d + half, d] = -1.0          # rot[d<64] = -q[d+64]
    for d in range(half, HD):
        perm[d - half, d] = 1.0           # rot[d>=64] = +q[d-64]

    mask = np.where(np.arange(HD)[None, :] >= np.arange(HD)[:, None],
                    0.0, NEG_BIG).astype(np.float32)  # [k, q]
    ones1 = np.ones((1, HD), dtype=np.float32)
    onesk = np.ones((HD, 1), dtype=ml_dtypes.bfloat16)
    ident = np.eye(HD, dtype=np.float32)
    return dict(ropeA=ropeA, ropeB=ropeB, perm=perm, mask=mask,
                ones1=ones1, onesk=onesk, ident=ident)


def host_in_maps(cfg, x, Wqkv, Wout):
    """Build per-core input maps from the full tensors."""
    B, L, D, QH, HD = cfg["B"], cfg["L"], cfg["D"], cfg["QH"], cfg["HD"]
    nco = cfg["ncores"]
    tabs = host_tables(cfg)
    xT = np.ascontiguousarray(
        x.reshape(B * L, D).T.astype(np.float32))
    NHT = QH * nco      # total q heads
    in_maps = []
    for c in range(nco):
        q_rows = np.arange(c * QH * HD, (c + 1) * QH * HD)
        k_rows = np.arange(NHT * HD + c * HD, NHT * HD + (c + 1) * HD)
        v_rows = np.arange(NHT * HD + nco * HD + c * HD,
                           NHT * HD + nco * HD + (c + 1) * HD)
        rows = np.concatenate([q_rows, k_rows, v_rows])
        wq = np.ascontiguousarray(Wqkv[rows, :].T.astype(np.float32))
        import ml_dtypes
        cols = np.arange(c * QH * HD, (c + 1) * QH * HD)
        wo = np.ascontiguousarray(Wout[:, cols].T.astype(ml_dtypes.bfloat16))
        m = dict(xT=xT, wq=wq, wo=wo)
        m.update(tabs)
        in_maps.append(m)
    return in_maps


_PROGRAM_CACHE = {}
LAST_RESULTS = None


def _get_program(cfg_key, cfg):
    if cfg_key not in _PROGRAM_CACHE:
        _PROGRAM_CACHE[cfg_key] = build_program(cfg)
    return _PROGRAM_CACHE[cfg_key]


def kernel(x, Wqkv, Wout):
    cfg = default_cfg()
    B, L, D = cfg["B"], cfg["L"], cfg["D"]
    x = np.asarray(x, dtype=np.float32)
    Wqkv = np.asarray(Wqkv, dtype=np.float32)
    Wout = np.asarray(Wout, dtype=np.float32)
    nc = _get_program("full", cfg)
    in_maps = host_in_maps(cfg, x, Wqkv, Wout)
    res = bass_utils.run_bass_kernel_spmd(
        nc, in_maps, core_ids=list(range(cfg["ncores"])))
    global LAST_RESULTS
    LAST_RESULTS = res
    acc = np.zeros((B * L, D), dtype=np.float64)
    for c in range(cfg["ncores"]):
        acc += np.asarray(res.results[c]["out"], dtype=np.float32)
    return acc.astype(np.float32).reshape(B, L, D)


# ---------------------------------------------------------------------------
# dev helpers (not used by the grading harness)

def _np_partial_reference(cfg, x, Wqkv_rows, Wout_cols_T):
    """Numpy reference for ONE core's partial output.

    Wqkv_rows: [R, D] (q heads, k, v rows for this core)
    Wout_cols_T: [QH*HD, D] (transposed slice of Wout columns)
    """
    B, L, D, QH, HD = cfg["B"], cfg["L"], cfg["D"], cfg["QH"], cfg["HD"]
    CLIP, theta = cfg["CLIP"], cfg["theta"]
    half = HD // 2
    xf = x.reshape(B * L, D).astype(np.float64)
    qkv = np.clip(xf @ Wqkv_rows.astype(np.float64).T, -CLIP, CLIP)
    qkv = qkv.reshape(B, L, (QH + 2), HD)
    q = qkv[:, :, :QH, :]            # [B, L, QH, HD]
    k = qkv[:, :, QH, :]             # [B, L, HD]
    v = qkv[:, :, QH + 1, :]         # [B, L, HD]

    inv_freq = 1.0 / (theta ** (np.arange(half, dtype=np.float64) / half))
    ang = np.arange(L, dtype=np.float64)[:, None] * inv_freq[None, :]
    cos, sin = np.cos(ang), np.sin(ang)      # [L, half]

    def rope(t):  # t [B, L, ..., HD] with positions on axis 1
        t1, t2 = t[..., :half], t[..., half:]
        shape = [1, L] + [1] * (t.ndim - 3) + [half]
        c = cos.reshape(L, half).reshape(shape)
        s = sin.reshape(L, half).reshape(shape)
        return np.concatenate([t1 * c - t2 * s, t2 * c + t1 * s], axis=-1)

    q = rope(q)
    k = rope(k)
    scalev = 1.0 / math.sqrt(HD)
    causal = np.tril(np.ones((L, L), dtype=bool))
    outs = []
    for bi in range(B):
        heads = []
        for h in range(QH):
            s = (q[bi, :, h, :] @ k[bi].T) * scalev
            s = np.where(causal, s, -np.inf)
            p = np.exp(s - s.max(axis=-1, keepdims=True))
            p /= p.sum(axis=-1, keepdims=True)
            heads.append(p @ v[bi])
        attn = np.concatenate(heads, axis=-1)     # [L, QH*HD]
        outs.append(attn)
    attn = np.stack(outs, 0).reshape(B * L, QH * HD)
    return (attn @ Wout_cols_T.astype(np.float64)).astype(np.float32)


def _mini_test(mode="sim"):
    from concourse.bass_interp import CoreSim
    cfg = mini_cfg()
    B, L, D, QH, HD = cfg["B"], cfg["L"], cfg["D"], cfg["QH"], cfg["HD"]
    R = (QH + 2) * HD
    rng = np.random.default_rng(0)
    x = (rng.standard_normal((B, L, D)) * 1.0).astype(np.float32)
    Wqkv_rows = (rng.standard_normal((R, D)) * D ** -0.5).astype(np.float32)
    WoT = (rng.standard_normal((QH * HD, D)) * D ** -0.5).astype(np.float32)

    nc = build_program(cfg)
    tabs = host_tables(cfg)
    xT = np.ascontiguousarray(x.reshape(B * L, D).T)
    wq = np.ascontiguousarray(Wqkv_rows.T)
    import ml_dtypes
    in_map = dict(xT=xT, wq=wq, wo=WoT.astype(ml_dtypes.bfloat16))
    in_map.update(tabs)

    want = _np_partial_reference(cfg, x, Wqkv_rows, WoT)

    if mode == "sim":
        sim = CoreSim(nc, trace=False)
        for kk, vv in in_map.items():
            sim.tensor(kk)[:] = vv
        sim.simulate(check_with_hw=False)
        got = np.asarray(sim.tensor("out"), dtype=np.float32)
    else:
        res = bass_utils.run_bass_kernel_spmd(nc, [in_map], core_ids=[0])
        got = np.asarray(res.results[0]["out"], dtype=np.float32)
    relmax = np.abs(got - want).max() / np.abs(want).max()
    rel2 = np.linalg.norm(got - want) / np.linalg.norm(want)
    print(f"mini {mode}: relmax={relmax:.3e} rel2={rel2:.3e}")


if __name__ == "__main__":
    _mini_test(sys.argv[1] if len(sys.argv) > 1 else "sim")
